# revision 2
# baseline (speedup 1.0000x reference)
"""Trainium2 Bass kernel for nn_LoadPathLoss — v3 (PE shifts + stripped sems).

reference computation:
  structure = state[:, ch]                  # [B=4, D=64, H=128, W=128]
  s = structure[:, 0]
  for z in 1..63:  s = max(s, min(structure[:, z], maxpool3x3(s)))
  return relu(structure - s[:, None]).mean()

Strategy (8 cores = 4 batches x 2 W-halves):
  - Each batch is W-split across a core pair; the right-half core receives
    the image flipped in W so both cores run the identical left-aligned
    kernel (maxpool is flip-invariant).  Columns shrink with the light cone:
    step z computes cols [0, 64+(63-z)) — no halo exchange needed.
  - The H-direction (partition) shifts use two fp16 PE shift-matmuls into
    PSUM (identity shifted by +/-1; zero boundary fill).  Values carry a +8
    offset so psum zero-fill and the W-pad zeros act as -inf.  Dummy matmuls
    keep the PE p-state warm so the real shifts run at high clock.
  - Per step: A=max(up,dn) reads PSUM once; hm=max(A,S) into a W-padded
    tile; two column maxes; min with m.  m=max(c_z+8, S) runs on GpSimd,
    hidden under the DVE chain.  All chain tiles fp16 (2x DVE mode).
  - After Tile compiles, same-engine semaphore waits are stripped: engines
    execute their streams in order and the DVE pipeline drain enforces the
    output hazard between back-to-back ops, so only cross-engine and
    DMA-completion waits are kept.
  - Phase 2 reduces max(c, s) over the owned half's cols with per-partition
    accumulators, split across DVE and GpSimd; host combines in f64.
"""

import numpy as np

B, C, D, H, W = 4, 8, 64, 128, 128
NCORES = 8
ZCHUNK = 8
NCHUNK = D // ZCHUNK
SHIFT = 8.0

_cached = {}


def _strip_same_engine_waits(nc):
    """Remove semaphore waits that only order instructions within one engine.

    A sem is strippable for engine E when every update to it comes from a
    synchronous compute instruction on E.  Engines execute their instruction
    stream in order and the DVE pipeline drain enforces write-before-read
    between back-to-back ops, so those waits are redundant; DMA-completion
    sems (async updates) and cross-engine waits are kept.
    """
    import concourse.mybir as mybir

    DMA_LIKE = {
        "DMACopy", "DmaTransposeAnt", "DMAGatherAnt", "DMAScatterAddAnt",
        "KVWritebackAnt", "PagedWritebackAnt", "TriggerDma", "CollectiveCompute",
        "RemoteDMADescs", "RemoteDMABroadcastDescs", "RemoteDMAFusedDescs",
    }
    fn = nc.m.functions[0]
    insts = [i for blk in fn.blocks for i in blk.instructions]
    updaters = {}
    for inst in insts:
        si = inst.sync_info
        if si is None:
            continue
        for u in si.on_update:
            updaters.setdefault(u.id, []).append((inst.engine, str(inst.opcode)))
    strippable = {}
    for sid, ups in updaters.items():
        engines = {e for e, _ in ups}
        opcodes = {o for _, o in ups}
        if len(engines) == 1 and not (opcodes & DMA_LIKE):
            strippable[sid] = next(iter(engines))
    n_stripped = 0
    for inst in insts:
        si = inst.sync_info
        if si is None or not si.on_wait:
            continue
        keep = [w for w in si.on_wait if strippable.get(w.id) != inst.engine]
        if len(keep) != len(si.on_wait):
            n_stripped += len(si.on_wait) - len(keep)
            try:
                si.on_wait = keep
            except Exception:
                inst.sync_info = mybir.SyncInfo(on_wait=keep, on_update=list(si.on_update))
    return n_stripped


def _build_nc(dt16=True, strip=True, m_on_gps=False, n_gps_phase2=0,
              n_dummy=0, dummy_w=256, d_steps=D, do_phase2=True):
    import concourse.bacc as bacc
    import concourse.mybir as mybir
    from concourse.tile import TileContext

    fp32 = mybir.dt.float32
    fp16 = mybir.dt.float16
    cdt = fp16 if dt16 else fp32
    mx = mybir.AluOpType.max
    mn = mybir.AluOpType.min
    add = mybir.AluOpType.add

    nc = bacc.Bacc("TRN2", target_bir_lowering=False, debug=False)
    cb = nc.dram_tensor("cb", [D, H, W], fp32, kind="ExternalInput")
    shifts = nc.dram_tensor("shifts", [H, 2 * H], cdt, kind="ExternalInput")
    out = nc.dram_tensor("out", [H, NCHUNK + 1], fp32, kind="ExternalOutput")

    with TileContext(nc) as tc:
        with (
            tc.tile_pool(name="sbuf", bufs=1) as pool,
            tc.tile_pool(name="psum", bufs=2, space="PSUM") as psum,
            tc.tile_pool(name="psumd", bufs=1, space="PSUM") as psumd,
        ):
            sh0 = pool.tile([H, 2 * H], cdt, tag="sh0")
            sh = pool.tile([H, 2 * H], cdt, tag="sh")
            c01 = pool.tile([H, 2, W], fp32, tag="c01")
            chunks = [
                pool.tile([H, ZCHUNK, W], fp32, tag=f"cb{k}", name=f"cb{k}")
                for k in range(NCHUNK)
            ]
            S = pool.tile([H, W], cdt, tag="S")
            A = pool.tile([H, W + 1], cdt, tag="A")
            P = pool.tile([H, W + 2], cdt, tag="P")
            t129 = pool.tile([H, W + 1], cdt, tag="t129")
            below = pool.tile([H, W], cdt, tag="below")
            m = pool.tile([H, W], cdt, tag="m")
            sraw = pool.tile([H, W // 2], fp32, tag="sraw")
            acc = pool.tile([H, NCHUNK + 1], fp32, tag="acc")
            dum = pool.tile([H, dummy_w], cdt, tag="dum")
            pd = psumd.tile([H, dummy_w], fp32, tag="pd")

            # DMAs: weights first, then z0/z1 (gates S init and step 1), chunks
            nc.sync.dma_start(out=sh0[:], in_=shifts[:, :])
            nc.sync.dma_start(out=c01[:], in_=cb[0:2].rearrange("z h w -> h z w"))
            for k in range(NCHUNK):
                src = cb[k * ZCHUNK : (k + 1) * ZCHUNK].rearrange("z h w -> h z w")
                nc.sync.dma_start(out=chunks[k][:], in_=src)

            # weights: DVE-copy shield so matmuls wait on DVE only
            nc.vector.tensor_copy(sh[:], sh0[:])
            nc.vector.memset(dum[:], 0.0)
            nc.vector.memset(P[:], 0.0)   # pad cols 0 and W+1 stay 0 = -inf

            # S init: cols [0, 127) of z=0, shifted +8
            V0 = 64 + (D - 1)
            nc.vector.tensor_scalar_add(S[:, 0:V0], c01[:, 0, :][:, 0:V0], SHIFT)

            geng = nc.gpsimd if m_on_gps else nc.vector
            for z in range(1, d_steps):
                k, j = z // ZCHUNK, z % ZCHUNK
                V = 64 + (D - 1 - z)
                csrc = c01[:, 1, :] if z == 1 else chunks[k][:, j, :]
                # m = max(c_z + 8, S) off-chain
                geng.scalar_tensor_tensor(
                    out=m[:, 0:V], in0=csrc[:, 0:V], scalar=SHIFT,
                    in1=S[:, 0:V], op0=add, op1=mx,
                )
                # PE: up/dn partition shifts into psum (fp16, zero boundary)
                ps = psum.tile([H, 2 * W], fp32, tag="ps", name=f"ps{z}")
                nc.tensor.matmul(
                    out=ps[:, 0 : V + 1], lhsT=sh[:, 0:H], rhs=S[:, 0 : V + 1],
                    start=True, stop=True,
                )
                nc.tensor.matmul(
                    out=ps[:, W : W + V + 1], lhsT=sh[:, H : 2 * H],
                    rhs=S[:, 0 : V + 1], start=True, stop=True,
                )
                # keep PE warm
                for dd in range(n_dummy):
                    nc.tensor.matmul(
                        out=pd[:], lhsT=sh[:, 0:H], rhs=dum[:],
                        start=True, stop=True,
                    )
                # H-combine: each op reads exactly one PSUM input
                nc.vector.tensor_tensor(
                    out=A[:, 0 : V + 1], in0=ps[:, 0 : V + 1],
                    in1=S[:, 0 : V + 1], op=mx,
                )
                nc.vector.tensor_tensor(
                    out=P[:, 1 : V + 2], in0=A[:, 0 : V + 1],
                    in1=ps[:, W : W + V + 1], op=mx,
                )
                nc.vector.tensor_tensor(
                    out=t129[:, 0 : V + 1], in0=P[:, 0 : V + 1], in1=P[:, 1 : V + 2],
                    op=mx,
                )
                nc.vector.tensor_tensor(
                    out=below[:, 0:V], in0=t129[:, 0:V], in1=P[:, 2 : V + 2], op=mx
                )
                nc.vector.tensor_tensor(
                    out=S[:, 0:V], in0=below[:, 0:V], in1=m[:, 0:V], op=mn
                )

            # phase 2 over owned cols [0, 64): acc[:,k] = sum_{j,w} max(c, s);
            # acc[:, NCHUNK] = row sums of s
            HW2 = W // 2
            nc.vector.tensor_scalar_add(sraw[:], S[:, 0:HW2], -SHIFT)
            nc.vector.tensor_reduce(
                out=acc[:, NCHUNK : NCHUNK + 1], in_=sraw[:],
                axis=mybir.AxisListType.X, op=add,
            )
            sbc = sraw[:].unsqueeze(1).broadcast_to((H, ZCHUNK, HW2))
            for k in range(NCHUNK if do_phase2 else 0):
                eng = nc.vector
                eng.scalar_tensor_tensor(
                    out=chunks[k][:, :, 0:HW2], in0=chunks[k][:, :, 0:HW2],
                    scalar=0.0, in1=sbc, op0=mybir.AluOpType.bypass, op1=mx,
                    accum_out=acc[:, k : k + 1],
                )

            nc.sync.dma_start(out=out[:, :], in_=acc[:])

    nc.compile()
    if strip:
        nc._n_stripped = _strip_same_engine_waits(nc)
    return nc


def _shift_mats(dt16=True):
    dt = np.float16 if dt16 else np.float32
    U = np.zeros((H, H), dtype=dt)   # lhsT: out[p] = x[p+1]
    Dm = np.zeros((H, H), dtype=dt)  # lhsT: out[p] = x[p-1]
    for p in range(H - 1):
        U[p + 1, p] = 1.0
        Dm[p, p + 1] = 1.0
    return np.concatenate([U, Dm], axis=1)


def _make_runner(nc):
    """Cached multi-core PJRT runner (mirrors bass2jax.run_bass_via_pjrt but
    keeps the jitted shard_map so repeat calls skip retrace/recompile)."""
    import jax
    from jax.sharding import Mesh, PartitionSpec
    from jax.experimental.shard_map import shard_map
    import concourse.mybir as mybir
    from concourse import bass2jax

    bass2jax.install_neuronx_cc_hook()

    partition_name = nc.partition_id_tensor.name if nc.partition_id_tensor else None
    in_names, out_names, out_avals, zero_outs = [], [], [], []
    for alloc in nc.m.functions[0].allocations:
        if not isinstance(alloc, mybir.MemoryLocationSet):
            continue
        name = alloc.memorylocations[0].name
        if alloc.kind == "ExternalInput":
            if name != partition_name:
                in_names.append(name)
        elif alloc.kind == "ExternalOutput":
            shape = tuple(alloc.tensor_shape)
            dtype = mybir.dt.np(alloc.dtype)
            out_names.append(name)
            out_avals.append(jax.core.ShapedArray(shape, dtype))
            zero_outs.append(np.zeros(shape, dtype))
    n_params = len(in_names)
    n_outs = len(out_avals)
    all_names = in_names + out_names
    donate = tuple(range(n_params, n_params + n_outs))

    def _body(*args):
        operands = list(args)
        if partition_name is not None:
            operands.append(bass2jax.partition_id_tensor())
        outs = bass2jax._bass_exec_p.bind(
            *operands,
            out_avals=tuple(out_avals),
            in_names=tuple(all_names + ([partition_name] if partition_name else [])),
            out_names=tuple(out_names),
            lowering_input_output_aliases=(),
            sim_require_finite=True,
            sim_require_nnan=True,
            nc=nc,
        )
        return tuple(outs)

    devices = jax.devices()[:NCORES]
    mesh = Mesh(np.asarray(devices), ("core",))
    in_specs = (PartitionSpec("core"),) * (n_params + n_outs)
    out_specs = (PartitionSpec("core"),) * n_outs
    sharded = jax.jit(
        shard_map(_body, mesh=mesh, in_specs=in_specs, out_specs=out_specs,
                  check_rep=False),
        donate_argnums=donate, keep_unused=True,
    )

    def run(in_maps):
        args = [
            np.concatenate([np.asarray(mp[name]) for mp in in_maps], axis=0)
            for name in in_names
        ]
        zouts = [np.concatenate([z] * NCORES, axis=0) for z in zero_outs]
        outs = sharded(*args, *zouts)
        res = []
        for b in range(NCORES):
            d = {}
            for i, name in enumerate(out_names):
                full = np.asarray(outs[i])
                per = full.shape[0] // NCORES
                d[name] = full[b * per : (b + 1) * per]
            res.append(d)
        return res

    return run


def kernel(state, ch_structure):
    if "nc" not in _cached:
        _cached["nc"] = _build_nc()
        _cached["run"] = _make_runner(_cached["nc"])

    structure = np.ascontiguousarray(state[:, int(ch_structure)], dtype=np.float32)
    sh = _shift_mats()
    in_maps = []
    for b in range(B):
        left = structure[b]
        right = np.ascontiguousarray(left[:, :, ::-1])
        in_maps.append({"cb": left, "shifts": sh})
        in_maps.append({"cb": right, "shifts": sh})
    results = _cached["run"](in_maps)
    _cached["last"] = results

    total_max = 0.0
    total_s = 0.0
    for i in range(NCORES):
        o = results[i]["out"].astype(np.float64)
        total_max += o[:, :NCHUNK].sum()
        total_s += o[:, NCHUNK].sum()
    mean = (total_max - float(D) * total_s) / float(B * D * H * W)
    return np.asarray(mean, dtype=np.float32)


if __name__ == "__main__":
    rng = np.random.default_rng(0)
    st = rng.standard_normal((B, C, D, H, W)).astype(np.float32)
    print(kernel(st, 3))


# revision 3
# speedup vs baseline: 1000.0000x; 1000.0000x over previous
"""Trainium2 Bass kernel for nn_LoadPathLoss — v3 (PE shifts + stripped sems).

reference computation:
  structure = state[:, ch]                  # [B=4, D=64, H=128, W=128]
  s = structure[:, 0]
  for z in 1..63:  s = max(s, min(structure[:, z], maxpool3x3(s)))
  return relu(structure - s[:, None]).mean()

Strategy (8 cores = 4 batches x 2 W-halves):
  - Each batch is W-split across a core pair; the right-half core receives
    the image flipped in W so both cores run the identical left-aligned
    kernel (maxpool is flip-invariant).  Columns shrink with the light cone:
    step z computes cols [0, 64+(63-z)) — no halo exchange needed.
  - The H-direction (partition) shifts use two fp16 PE shift-matmuls into
    PSUM (identity shifted by +/-1; zero boundary fill).  Values carry a +8
    offset so psum zero-fill and the W-pad zeros act as -inf (BIR forbids
    compute-engine access patterns starting at unaligned partitions, so the
    row shifts cannot be done with partition-offset DVE ops).
  - Per step, 6 DVE ops: m=max(c_z+8, S) runs first and fills the idle gap
    while the PE round trip (sem + matmuls + PSUM latency) completes; then
    h1=max(up_psum, S), hm=max(h1, dn_psum) into a W-padded tile (each op
    reads exactly one PSUM operand), two column maxes, and min with m.
    All chain tiles fp16 (2x DVE mode).
  - After Tile compiles, same-engine semaphore waits are stripped: engines
    execute their streams in order and the DVE pipeline drain enforces the
    output hazard between back-to-back ops, so only cross-engine and
    DMA-completion waits are kept.
  - Phase 2 reduces max(c, s) over the owned half's cols with per-partition
    accumulators on DVE; host combines partials in f64.
"""

import numpy as np

B, C, D, H, W = 4, 8, 64, 128, 128
NCORES = 8
ZCHUNK = 8
NCHUNK = D // ZCHUNK
SHIFT = 8.0

_cached = {}


def _strip_same_engine_waits(nc):
    """Remove semaphore waits that only order instructions within one engine.

    A sem is strippable for engine E when every update to it comes from a
    synchronous compute instruction on E.  Engines execute their instruction
    stream in order and the DVE pipeline drain enforces write-before-read
    between back-to-back ops, so those waits are redundant; DMA-completion
    sems (async updates) and cross-engine waits are kept.
    """
    import concourse.mybir as mybir

    DMA_LIKE = {
        "DMACopy", "DmaTransposeAnt", "DMAGatherAnt", "DMAScatterAddAnt",
        "KVWritebackAnt", "PagedWritebackAnt", "TriggerDma", "CollectiveCompute",
        "RemoteDMADescs", "RemoteDMABroadcastDescs", "RemoteDMAFusedDescs",
    }
    fn = nc.m.functions[0]
    insts = [i for blk in fn.blocks for i in blk.instructions]
    updaters = {}
    for inst in insts:
        si = inst.sync_info
        if si is None:
            continue
        for u in si.on_update:
            updaters.setdefault(u.id, []).append((inst.engine, str(inst.opcode)))
    strippable = {}
    for sid, ups in updaters.items():
        engines = {e for e, _ in ups}
        opcodes = {o for _, o in ups}
        if len(engines) == 1 and not (opcodes & DMA_LIKE):
            strippable[sid] = next(iter(engines))
    n_stripped = 0
    for inst in insts:
        si = inst.sync_info
        if si is None or not si.on_wait:
            continue
        keep = [w for w in si.on_wait if strippable.get(w.id) != inst.engine]
        if len(keep) != len(si.on_wait):
            n_stripped += len(si.on_wait) - len(keep)
            try:
                si.on_wait = keep
            except Exception:
                inst.sync_info = mybir.SyncInfo(on_wait=keep, on_update=list(si.on_update))
    return n_stripped


def _build_nc(dt16=True, strip=True, m_on_gps=False, n_gps_phase2=0,
              n_dummy=0, dummy_w=256, d_steps=D, do_phase2=True):
    import concourse.bacc as bacc
    import concourse.mybir as mybir
    from concourse.tile import TileContext

    fp32 = mybir.dt.float32
    fp16 = mybir.dt.float16
    cdt = fp16 if dt16 else fp32
    mx = mybir.AluOpType.max
    mn = mybir.AluOpType.min
    add = mybir.AluOpType.add

    nc = bacc.Bacc("TRN2", target_bir_lowering=False, debug=False)
    cb = nc.dram_tensor("cb", [D, H, W], fp32, kind="ExternalInput")
    shifts = nc.dram_tensor("shifts", [H, 2 * H], cdt, kind="ExternalInput")
    out = nc.dram_tensor("out", [H, NCHUNK + 1], fp32, kind="ExternalOutput")

    with TileContext(nc) as tc:
        with (
            tc.tile_pool(name="sbuf", bufs=1) as pool,
            tc.tile_pool(name="psum", bufs=2, space="PSUM") as psum,
            tc.tile_pool(name="psumd", bufs=1, space="PSUM") as psumd,
        ):
            sh0 = pool.tile([H, 2 * H], cdt, tag="sh0")
            sh = pool.tile([H, 2 * H], cdt, tag="sh")
            c01 = pool.tile([H, 2, W], fp32, tag="c01")
            chunks = [
                pool.tile([H, ZCHUNK, W], fp32, tag=f"cb{k}", name=f"cb{k}")
                for k in range(NCHUNK)
            ]
            S = pool.tile([H, W], cdt, tag="S")
            A = pool.tile([H, W + 1], cdt, tag="A")
            P = pool.tile([H, W + 2], cdt, tag="P")
            t129 = pool.tile([H, W + 1], cdt, tag="t129")
            below = pool.tile([H, W], cdt, tag="below")
            m = pool.tile([H, W], cdt, tag="m")
            sraw = pool.tile([H, W // 2], fp32, tag="sraw")
            acc = pool.tile([H, NCHUNK + 1], fp32, tag="acc")
            dum = pool.tile([H, dummy_w], cdt, tag="dum")
            pd = psumd.tile([H, dummy_w], fp32, tag="pd")

            # DMAs: weights first, then z0/z1 (gates S init and step 1), chunks
            nc.sync.dma_start(out=sh0[:], in_=shifts[:, :])
            nc.sync.dma_start(out=c01[:], in_=cb[0:2].rearrange("z h w -> h z w"))
            for k in range(NCHUNK):
                src = cb[k * ZCHUNK : (k + 1) * ZCHUNK].rearrange("z h w -> h z w")
                nc.sync.dma_start(out=chunks[k][:], in_=src)

            # weights: DVE-copy shield so matmuls wait on DVE only
            nc.vector.tensor_copy(sh[:], sh0[:])
            nc.vector.memset(dum[:], 0.0)
            nc.vector.memset(P[:], 0.0)   # pad cols 0 and W+1 stay 0 = -inf

            # S init: cols [0, 127) of z=0, shifted +8
            V0 = 64 + (D - 1)
            nc.vector.tensor_scalar_add(S[:, 0:V0], c01[:, 0, :][:, 0:V0], SHIFT)

            geng = nc.gpsimd if m_on_gps else nc.vector
            for z in range(1, d_steps):
                k, j = z // ZCHUNK, z % ZCHUNK
                V = 64 + (D - 1 - z)
                csrc = c01[:, 1, :] if z == 1 else chunks[k][:, j, :]
                # m = max(c_z + 8, S) off-chain
                geng.scalar_tensor_tensor(
                    out=m[:, 0:V], in0=csrc[:, 0:V], scalar=SHIFT,
                    in1=S[:, 0:V], op0=add, op1=mx,
                )
                # PE: up/dn partition shifts into psum (fp16, zero boundary)
                ps = psum.tile([H, 2 * W], fp32, tag="ps", name=f"ps{z}")
                nc.tensor.matmul(
                    out=ps[:, 0 : V + 1], lhsT=sh[:, 0:H], rhs=S[:, 0 : V + 1],
                    start=True, stop=True,
                )
                nc.tensor.matmul(
                    out=ps[:, W : W + V + 1], lhsT=sh[:, H : 2 * H],
                    rhs=S[:, 0 : V + 1], start=True, stop=True,
                )
                # keep PE warm
                for dd in range(n_dummy):
                    nc.tensor.matmul(
                        out=pd[:], lhsT=sh[:, 0:H], rhs=dum[:],
                        start=True, stop=True,
                    )
                # H-combine: each op reads exactly one PSUM input
                nc.vector.tensor_tensor(
                    out=A[:, 0 : V + 1], in0=ps[:, 0 : V + 1],
                    in1=S[:, 0 : V + 1], op=mx,
                )
                nc.vector.tensor_tensor(
                    out=P[:, 1 : V + 2], in0=A[:, 0 : V + 1],
                    in1=ps[:, W : W + V + 1], op=mx,
                )
                nc.vector.tensor_tensor(
                    out=t129[:, 0 : V + 1], in0=P[:, 0 : V + 1], in1=P[:, 1 : V + 2],
                    op=mx,
                )
                nc.vector.tensor_tensor(
                    out=below[:, 0:V], in0=t129[:, 0:V], in1=P[:, 2 : V + 2], op=mx
                )
                nc.vector.tensor_tensor(
                    out=S[:, 0:V], in0=below[:, 0:V], in1=m[:, 0:V], op=mn
                )

            # phase 2 over owned cols [0, 64): acc[:,k] = sum_{j,w} max(c, s);
            # acc[:, NCHUNK] = row sums of s
            HW2 = W // 2
            nc.vector.tensor_scalar_add(sraw[:], S[:, 0:HW2], -SHIFT)
            nc.vector.tensor_reduce(
                out=acc[:, NCHUNK : NCHUNK + 1], in_=sraw[:],
                axis=mybir.AxisListType.X, op=add,
            )
            sbc = sraw[:].unsqueeze(1).broadcast_to((H, ZCHUNK, HW2))
            for k in range(NCHUNK if do_phase2 else 0):
                nc.vector.scalar_tensor_tensor(
                    out=chunks[k][:, :, 0:HW2], in0=chunks[k][:, :, 0:HW2],
                    scalar=0.0, in1=sbc, op0=mybir.AluOpType.bypass, op1=mx,
                    accum_out=acc[:, k : k + 1],
                )

            nc.sync.dma_start(out=out[:, :], in_=acc[:])

    nc.compile()
    if strip:
        nc._n_stripped = _strip_same_engine_waits(nc)
    return nc


def _shift_mats(dt16=True):
    dt = np.float16 if dt16 else np.float32
    U = np.zeros((H, H), dtype=dt)   # lhsT: out[p] = x[p+1]
    Dm = np.zeros((H, H), dtype=dt)  # lhsT: out[p] = x[p-1]
    for p in range(H - 1):
        U[p + 1, p] = 1.0
        Dm[p, p + 1] = 1.0
    return np.concatenate([U, Dm], axis=1)


def _make_runner(nc):
    """Cached multi-core PJRT runner (mirrors bass2jax.run_bass_via_pjrt but
    keeps the jitted shard_map so repeat calls skip retrace/recompile)."""
    import jax
    from jax.sharding import Mesh, PartitionSpec
    from jax.experimental.shard_map import shard_map
    import concourse.mybir as mybir
    from concourse import bass2jax

    bass2jax.install_neuronx_cc_hook()

    partition_name = nc.partition_id_tensor.name if nc.partition_id_tensor else None
    in_names, out_names, out_avals, zero_outs = [], [], [], []
    for alloc in nc.m.functions[0].allocations:
        if not isinstance(alloc, mybir.MemoryLocationSet):
            continue
        name = alloc.memorylocations[0].name
        if alloc.kind == "ExternalInput":
            if name != partition_name:
                in_names.append(name)
        elif alloc.kind == "ExternalOutput":
            shape = tuple(alloc.tensor_shape)
            dtype = mybir.dt.np(alloc.dtype)
            out_names.append(name)
            out_avals.append(jax.core.ShapedArray(shape, dtype))
            zero_outs.append(np.zeros(shape, dtype))
    n_params = len(in_names)
    n_outs = len(out_avals)
    all_names = in_names + out_names
    donate = tuple(range(n_params, n_params + n_outs))

    def _body(*args):
        operands = list(args)
        if partition_name is not None:
            operands.append(bass2jax.partition_id_tensor())
        outs = bass2jax._bass_exec_p.bind(
            *operands,
            out_avals=tuple(out_avals),
            in_names=tuple(all_names + ([partition_name] if partition_name else [])),
            out_names=tuple(out_names),
            lowering_input_output_aliases=(),
            sim_require_finite=True,
            sim_require_nnan=True,
            nc=nc,
        )
        return tuple(outs)

    devices = jax.devices()[:NCORES]
    mesh = Mesh(np.asarray(devices), ("core",))
    in_specs = (PartitionSpec("core"),) * (n_params + n_outs)
    out_specs = (PartitionSpec("core"),) * n_outs
    sharded = jax.jit(
        shard_map(_body, mesh=mesh, in_specs=in_specs, out_specs=out_specs,
                  check_rep=False),
        donate_argnums=donate, keep_unused=True,
    )

    def run(in_maps):
        args = [
            np.concatenate([np.asarray(mp[name]) for mp in in_maps], axis=0)
            for name in in_names
        ]
        zouts = [np.concatenate([z] * NCORES, axis=0) for z in zero_outs]
        outs = sharded(*args, *zouts)
        res = []
        for b in range(NCORES):
            d = {}
            for i, name in enumerate(out_names):
                full = np.asarray(outs[i])
                per = full.shape[0] // NCORES
                d[name] = full[b * per : (b + 1) * per]
            res.append(d)
        return res

    return run


def kernel(state, ch_structure):
    if "nc" not in _cached:
        _cached["nc"] = _build_nc()
        _cached["run"] = _make_runner(_cached["nc"])

    structure = np.ascontiguousarray(state[:, int(ch_structure)], dtype=np.float32)
    sh = _shift_mats()
    in_maps = []
    for b in range(B):
        left = structure[b]
        right = np.ascontiguousarray(left[:, :, ::-1])
        in_maps.append({"cb": left, "shifts": sh})
        in_maps.append({"cb": right, "shifts": sh})
    results = _cached["run"](in_maps)
    _cached["last"] = results

    total_max = 0.0
    total_s = 0.0
    for i in range(NCORES):
        o = results[i]["out"].astype(np.float64)
        total_max += o[:, :NCHUNK].sum()
        total_s += o[:, NCHUNK].sum()
    mean = (total_max - float(D) * total_s) / float(B * D * H * W)
    return np.asarray(mean, dtype=np.float32)


if __name__ == "__main__":
    rng = np.random.default_rng(0)
    st = rng.standard_normal((B, C, D, H, W)).astype(np.float32)
    print(kernel(st, 3))
